# revision 5
# baseline (speedup 1.0000x reference)
"""Multi-head attention forward (B=8, N=1024, DIM=768, H=12) on 8 TRN2 cores.

Data parallel: core c computes batch element c entirely locally (no
collectives). Layouts chosen so no probability-matrix transpose is needed:
  - q_t, k_t feature-major [dim, tok]  (2 heads per 128-partition tile)
  - v token-major [tok, head*(64+1)] with a ones column per head, so the
    AV matmul's psum row 64 accumulates the softmax denominator
  - scores computed transposed s_T[j, i]; softmax sum over j comes from the
    ones column; exp has no max-subtraction (scores ~ N(0,1), fp32-safe)
  - AV output is feature-major -> feeds proj lhsT directly
All matmuls run in float32r (TF32-like; 4x faster than fp32 on the PE).
"""

from contextlib import ExitStack

import numpy as np

import concourse.bacc as bacc
import concourse.mybir as mybir
import concourse.tile as tile
from concourse.bass_utils import run_bass_kernel_spmd
from concourse.masks import make_identity

N = 1024
DIM = 768
NH = 12
HD = 64
SCALE = HD ** -0.5
P = 128
KT = DIM // P          # 6 contraction tiles over dim
TT = N // P            # 8 token tiles
QKFT = 2 * DIM // P    # 12 feature tiles covering q then k
HD1 = HD + 1           # 65: v head slice + ones column
F32 = mybir.dt.float32
F32R = mybir.dt.float32r
EXP = mybir.ActivationFunctionType.Exp
N_CORES = 8


def _r(ap):
    return ap.bitcast(F32R)


def build_attention_nc():
    nc = bacc.Bacc(None, target_bir_lowering=False)
    x_d = nc.declare_dram_parameter("x", [N, DIM], F32, isOutput=False)
    wqkv_d = nc.declare_dram_parameter("w_qkv", [DIM, 3 * DIM], F32, isOutput=False)
    bqkv_d = nc.declare_dram_parameter("b_qkv", [3 * DIM], F32, isOutput=False)
    wproj_d = nc.declare_dram_parameter("w_proj", [DIM, DIM], F32, isOutput=False)
    bproj_d = nc.declare_dram_parameter("b_proj", [DIM], F32, isOutput=False)
    out_d = nc.declare_dram_parameter("out", [N, DIM], F32, isOutput=True)

    with tile.TileContext(nc) as tc, ExitStack() as ctx:
        const = ctx.enter_context(tc.tile_pool(name="const", bufs=1))
        ident = const.tile([P, P], F32)
        make_identity(nc, ident[:])

        # biases: q/k per-partition [128, ft]; v and proj broadcast to [128, 768]
        bqk = const.tile([P, QKFT], F32)
        nc.sync.dma_start(bqk[:], bqkv_d[0 : 2 * DIM].rearrange("(f p) -> p f", p=P))
        bv_row = const.tile([1, DIM], F32)
        nc.sync.dma_start(bv_row[:], bqkv_d[2 * DIM : 3 * DIM].rearrange("(a d) -> a d", a=1))
        bv_bc = const.tile([P, DIM], F32)
        nc.gpsimd.partition_broadcast(bv_bc[:], bv_row[:])
        bp_row = const.tile([1, DIM], F32)
        nc.sync.dma_start(bp_row[:], bproj_d[:].rearrange("(a d) -> a d", a=1))
        bp_bc = const.tile([P, DIM], F32)
        nc.gpsimd.partition_broadcast(bp_bc[:], bp_row[:])

        # resident intermediates
        qkt_pool = ctx.enter_context(tc.tile_pool(name="qkt", bufs=QKFT))
        qk_t = [qkt_pool.tile([P, N], F32R, tag="qkt", name=f"qkt{i}") for i in range(QKFT)]
        v_pool = ctx.enter_context(tc.tile_pool(name="vsb", bufs=TT))
        v_sb = [v_pool.tile([P, NH * HD1], F32R, tag="vsb", name=f"vsb{i}") for i in range(TT)]
        at_pool = ctx.enter_context(tc.tile_pool(name="attnt", bufs=KT))
        attn_t = [at_pool.tile([P, N], F32R, tag="attnt", name=f"attnt{i}") for i in range(KT)]
        # proj weights fetched early
        wp_pool = ctx.enter_context(tc.tile_pool(name="wp", bufs=KT))
        w_p = [wp_pool.tile([P, DIM], F32R, tag="wp", name=f"wp{i}") for i in range(KT)]
        for kc in range(KT):
            nc.sync.dma_start(w_p[kc][:], _r(wproj_d[kc * P : (kc + 1) * P, :]))

        # ---------------- phase 1+2: x^T, qkv ----------------
        with (
            tc.tile_pool(name="xt", bufs=KT) as xt_pool,
            tc.tile_pool(name="xnat", bufs=3) as xnat_pool,
            tc.tile_pool(name="trps", bufs=2, space="PSUM") as trps_pool,
            tc.tile_pool(name="qkps", bufs=2, space="PSUM") as qkps_pool,
            tc.tile_pool(name="vps", bufs=2, space="PSUM") as vps_pool,
            tc.tile_pool(name="wqk", bufs=12) as wqk_pool,
            tc.tile_pool(name="wv", bufs=KT) as wv_pool,
        ):
            x_t = [xt_pool.tile([P, N], F32R, tag="xt", name=f"xt{i}") for i in range(KT)]
            for tt in range(TT):
                xn = xnat_pool.tile([P, DIM], F32, tag="xn")
                nc.sync.dma_start(xn[:], x_d[tt * P : (tt + 1) * P, :])
                for kc in range(KT):
                    pt = trps_pool.tile([P, P], F32, tag="trps")
                    nc.tensor.transpose(pt[:], xn[:, kc * P : (kc + 1) * P], ident[:])
                    nc.vector.tensor_copy(x_t[kc][:, tt * P : (tt + 1) * P], pt[:])

            # v = x @ w_v  (token-major), interleaved strided store + ones col
            w_v = [wv_pool.tile([P, DIM], F32R, tag="wv", name=f"wv{i}") for i in range(KT)]
            for kc in range(KT):
                nc.sync.dma_start(w_v[kc][:], _r(wqkv_d[kc * P : (kc + 1) * P, 2 * DIM : 3 * DIM]))
            for tt in range(TT):
                ps = vps_pool.tile([P, DIM], F32, tag="vps")
                for nh0, nh1 in ((0, 512), (512, 768)):
                    for kc in range(KT):
                        nc.tensor.matmul(
                            ps[:, nh0:nh1],
                            x_t[kc][:, tt * P : (tt + 1) * P],
                            w_v[kc][:, nh0:nh1],
                            start=(kc == 0),
                            stop=(kc == KT - 1),
                        )
                vdst = v_sb[tt][:].rearrange("p (h c) -> p h c", c=HD1)
                nc.vector.tensor_add(
                    vdst[:, :, 0:HD],
                    ps[:].rearrange("p (h c) -> p h c", c=HD),
                    bv_bc[:].rearrange("p (h c) -> p h c", c=HD),
                )
                nc.vector.tensor_scalar(
                    vdst[:, :, HD:HD1],
                    bv_bc[:, 0:NH].rearrange("p (h o) -> p h o", o=1),
                    0.0,
                    1.0,
                    op0=mybir.AluOpType.mult,
                    op1=mybir.AluOpType.add,
                )

            # q_t, k_t = (x @ w_{q,k})^T  (feature-major)
            for ft in range(QKFT):
                wts = []
                for kc in range(KT):
                    wt = wqk_pool.tile([P, P], F32R, tag="wqk", name=f"wqk{ft}_{kc}")
                    nc.sync.dma_start(
                        wt[:], _r(wqkv_d[kc * P : (kc + 1) * P, ft * P : (ft + 1) * P])
                    )
                    wts.append(wt)
                for ih in range(2):
                    ps = qkps_pool.tile([P, 512], F32, tag="qkps")
                    for kc in range(KT):
                        nc.tensor.matmul(
                            ps[:],
                            wts[kc][:],
                            x_t[kc][:, ih * 512 : (ih + 1) * 512],
                            start=(kc == 0),
                            stop=(kc == KT - 1),
                        )
                    nc.vector.tensor_scalar_add(
                        qk_t[ft][:, ih * 512 : (ih + 1) * 512], ps[:], bqk[:, ft : ft + 1]
                    )

        # ---------------- phase 3: attention ----------------
        with (
            tc.tile_pool(name="sps", bufs=2, space="PSUM") as sps_pool,
            tc.tile_pool(name="ops", bufs=2, space="PSUM") as ops_pool,
            tc.tile_pool(name="psb", bufs=3) as p_pool,
            tc.tile_pool(name="nrm", bufs=2) as nrm_pool,
        ):
            for h in range(NH):
                p0 = HD * (h % 2)
                qt = qk_t[h // 2]
                kt = qk_t[NH // 2 + h // 2]
                po = ops_pool.tile([HD1, N], F32, tag="ops")
                for jc in range(TT):
                    ps = sps_pool.tile([P, N], F32, tag="sps")
                    for ih in range(2):
                        nc.tensor.matmul(
                            ps[:, ih * 512 : (ih + 1) * 512],
                            kt[p0 : p0 + HD, jc * P : (jc + 1) * P],
                            qt[p0 : p0 + HD, ih * 512 : (ih + 1) * 512],
                            start=True,
                            stop=True,
                            tile_position=(p0, 0),
                        )
                    pe = p_pool.tile([P, N], F32R, tag="psb")
                    nc.scalar.activation(pe[:], ps[:], EXP, scale=SCALE)
                    for ih in range(2):
                        nc.tensor.matmul(
                            po[:, ih * 512 : (ih + 1) * 512],
                            v_sb[jc][:, h * HD1 : (h + 1) * HD1],
                            pe[:, ih * 512 : (ih + 1) * 512],
                            start=(jc == 0),
                            stop=(jc == TT - 1),
                        )
                rden = nrm_pool.tile([1, N], F32, tag="rden")
                nc.vector.reciprocal(rden[:], po[HD:HD1, :])
                rbc = nrm_pool.tile([HD, N], F32, tag="rbc")
                nc.gpsimd.partition_broadcast(rbc[:], rden[:])
                nc.vector.tensor_mul(attn_t[h // 2][p0 : p0 + HD, :], po[0:HD, :], rbc[:])

        # ---------------- phase 4: proj ----------------
        with (
            tc.tile_pool(name="yps", bufs=2, space="PSUM") as yps_pool,
            tc.tile_pool(name="ysb", bufs=3) as y_pool,
        ):
            for it in range(TT):
                ps = yps_pool.tile([P, DIM], F32, tag="yps")
                for nh0, nh1 in ((0, 512), (512, 768)):
                    for kc in range(KT):
                        nc.tensor.matmul(
                            ps[:, nh0:nh1],
                            attn_t[kc][:, it * P : (it + 1) * P],
                            w_p[kc][:, nh0:nh1],
                            start=(kc == 0),
                            stop=(kc == KT - 1),
                        )
                ysb = y_pool.tile([P, DIM], F32, tag="ysb")
                nc.vector.tensor_add(ysb[:], ps[:], bp_bc[:])
                nc.sync.dma_start(out_d[it * P : (it + 1) * P, :], ysb[:])

    nc.compile()
    return nc


_NC_CACHE = None


def _get_nc():
    global _NC_CACHE
    if _NC_CACHE is None:
        _NC_CACHE = build_attention_nc()
    return _NC_CACHE


def make_in_maps(inputs):
    x = np.ascontiguousarray(np.asarray(inputs["x"], dtype=np.float32))
    w_qkv = np.ascontiguousarray(np.asarray(inputs["w_qkv"], dtype=np.float32))
    b_qkv = np.ascontiguousarray(np.asarray(inputs["b_qkv"], dtype=np.float32))
    w_proj = np.ascontiguousarray(np.asarray(inputs["w_proj"], dtype=np.float32))
    b_proj = np.ascontiguousarray(np.asarray(inputs["b_proj"], dtype=np.float32))
    return [
        {"x": x[c], "w_qkv": w_qkv, "b_qkv": b_qkv, "w_proj": w_proj, "b_proj": b_proj}
        for c in range(N_CORES)
    ]


def kernel(**inputs) -> np.ndarray:
    nc = _get_nc()
    in_maps = make_in_maps(inputs)
    res = run_bass_kernel_spmd(nc, in_maps, core_ids=list(range(N_CORES)))
    return np.stack([res.results[c]["out"] for c in range(N_CORES)], axis=0)


# revision 27
# speedup vs baseline: 37.2995x; 37.2995x over previous
"""Multi-head attention forward (B=8, N=1024, DIM=768, H=12) on 8 TRN2 cores.

Data parallel: core c computes batch element c entirely locally (no
collectives). Layouts chosen so no probability-matrix transpose is needed:
  - q_t, k_t feature-major [dim, tok]  (2 heads per 128-partition tile)
  - v token-major [tok, head*(64+1)] with a ones column per head, so the
    AV matmul's psum row 64 accumulates the softmax denominator
  - scores computed transposed s_T[j, i]; softmax sum over j comes from the
    ones column; exp has no max-subtraction (scores ~ N(0,1), fp32-safe)
  - AV output is feature-major -> feeds proj lhsT directly
All matmuls run in float32r (TF32-like; 4x faster than fp32 on the PE).
The qk projection matmuls are drip-fed through the attention jc loop so the
ScalarE exp stream paces the kernel while the PE hides projection work.
"""

from contextlib import ExitStack

import numpy as np

import concourse.bacc as bacc
import concourse.mybir as mybir
import concourse.tile as tile
from concourse.bass_utils import run_bass_kernel_spmd
from concourse.masks import make_identity

N = 1024
DIM = 768
NH = 12
HD = 64
SCALE = HD ** -0.5
P = 128
KT = DIM // P          # 6 contraction tiles over dim
TT = N // P            # 8 token tiles
QKFT = 2 * DIM // P    # 12 feature tiles covering q then k
HD1 = HD + 1           # 65: v head slice + ones column
F32 = mybir.dt.float32
F32R = mybir.dt.float32r
BF16 = mybir.dt.bfloat16
EXP = mybir.ActivationFunctionType.Exp
N_CORES = 8


def _r(ap):
    return ap.bitcast(F32R)


def _emit_body(nc, tc, rep, dram, transpose_mode="pe", p_bufs=4, xq="sp"):
    x_d, wqkv_d, bqkv_d, wproj_d, bproj_d, out_d = dram

    with ExitStack() as body:
        const = body.enter_context(tc.tile_pool(name=f"const{rep}", bufs=1))
        # shared PSUM pools (8 banks):
        #  A: 2 x [128,1024] (4KB) slots - transposes, scores psum
        #  B: 2 x 4KB slots - v psum, qk psum, AV psum, proj psum
        psA = body.enter_context(tc.tile_pool(name=f"psA{rep}", bufs=2, space="PSUM"))
        psB = body.enter_context(tc.tile_pool(name=f"psB{rep}", bufs=2, space="PSUM"))

        # resident SBUF intermediates
        sb = body.enter_context(tc.tile_pool(name=f"sb{rep}", bufs=1))
        x_t = [sb.tile([P, N], BF16, name=f"xt{rep}_{i}") for i in range(KT)]
        qk_t = [sb.tile([P, N], BF16, name=f"qkt{rep}_{i}") for i in range(QKFT)]
        v_sb = [sb.tile([P, NH * HD1], BF16, name=f"vsb{rep}_{i}") for i in range(TT)]
        attn_t = [sb.tile([P, N], BF16, name=f"attnt{rep}_{i}") for i in range(KT)]

        if transpose_mode == "dma":
            # ------ x^T off the PE: cast to bf16 in DRAM, DMA-transpose in ---
            with tc.tile_pool(name=f"xbf{rep}", bufs=1, space="DRAM") as xbf_pool:
                xbf = xbf_pool.tile([N, DIM], BF16, name=f"xbf{rep}")
                nc.gpsimd.dma_start(xbf[:], x_d[:])
                for kc in range(KT):
                    nc.sync.dma_start_transpose(
                        x_t[kc][:], xbf[:, kc * P : (kc + 1) * P]
                    )
        else:
            # ------------ x^T on the PE (f32r transpose, bf16 copyback) ------
            ident_f = const.tile([P, P], F32, name=f"identf{rep}")
            make_identity(nc, ident_f[:])
            ident = const.tile([P, P], F32R, name=f"ident{rep}")
            nc.scalar.copy(ident[:], ident_f[:])
            with tc.tile_pool(name=f"xnat{rep}", bufs=3) as xnat_pool:
                for tt in range(TT):
                    xn = xnat_pool.tile([P, DIM], F32R, tag="xn", name=f"xn{rep}_{tt}")
                    xeng = nc.scalar if xq == "act" else nc.sync
                    xeng.dma_start(xn[:], _r(x_d[tt * P : (tt + 1) * P, :]))
                    for kc in range(KT):
                        pt = psA.tile([P, P], F32R, tag="psA", name=f"trp{rep}_{tt}_{kc}")
                        nc.tensor.transpose(pt[:], xn[:, kc * P : (kc + 1) * P], ident[:])
                        nc.vector.tensor_copy(x_t[kc][:, tt * P : (tt + 1) * P], pt[:])

        # biases: q/k per-partition [128, ft]; v and proj broadcast to [128, 768]
        bqk = const.tile([P, QKFT], F32, name=f"bqk{rep}")
        nc.sync.dma_start(bqk[:], bqkv_d[0 : 2 * DIM].rearrange("(f p) -> p f", p=P))
        bv_row = const.tile([1, DIM], F32, name=f"bvr{rep}")
        nc.sync.dma_start(bv_row[:], bqkv_d[2 * DIM : 3 * DIM].rearrange("(a d) -> a d", a=1))
        bv_bc = const.tile([P, DIM], F32, name=f"bvb{rep}")
        nc.gpsimd.partition_broadcast(bv_bc[:], bv_row[:])
        bp_row = const.tile([1, DIM], F32, name=f"bpr{rep}")
        nc.sync.dma_start(bp_row[:], bproj_d[:].rearrange("(a d) -> a d", a=1))
        bp_bc = const.tile([P, DIM], F32, name=f"bpb{rep}")
        nc.gpsimd.partition_broadcast(bp_bc[:], bp_row[:])

        # ------- attention pools (opened early so qk weights prime first) ----
        with (
            tc.tile_pool(name=f"wv{rep}", bufs=KT) as wv_pool,
            tc.tile_pool(name=f"wvs{rep}", bufs=3) as wvs_pool,
            tc.tile_pool(name=f"wqk{rep}", bufs=24) as wqk_pool,
            tc.tile_pool(name=f"wqs{rep}", bufs=12) as wqs_pool,
            tc.tile_pool(name=f"psb{rep}", bufs=p_bufs) as p_pool,
            tc.tile_pool(name=f"nrm{rep}", bufs=2) as nrm_pool,
            tc.tile_pool(name=f"stg{rep}", bufs=2) as stg_pool,
        ):
            w_qk = {}
            for ft in range(QKFT):
                w_qk[ft] = []
                for kc in range(KT):
                    wt = wqk_pool.tile([P, P], BF16, tag="wqk", name=f"wqk{rep}_{ft}_{kc}")
                    w_qk[ft].append(wt)

            def emit_wdma_pair(ftq):
                ftk = NH // 2 + ftq
                for kc in range(KT):
                    stg = wqs_pool.tile(
                        [P, 2, P], F32, tag="wqs", name=f"wqs{rep}_{ftq}_{kc}"
                    )
                    src_ap = wqkv_d[kc * P : (kc + 1) * P, :].rearrange(
                        "p (f c) -> p f c", c=P
                    )[:, ftq :: NH // 2, :][:, 0:2, :]
                    nc.sync.dma_start(stg[:], src_ap)
                    nc.vector.tensor_copy(w_qk[ftq][kc][:], stg[:, 0, :])
                    nc.vector.tensor_copy(w_qk[ftk][kc][:], stg[:, 1, :])

            # qk weights for the first feature-tile pair before w_v traffic
            emit_wdma_pair(0)

            # ---------------- v = x @ w_v (token-major + ones col) ------------
            w_v = [wv_pool.tile([P, DIM], BF16, tag="wv", name=f"wv{rep}_{i}") for i in range(KT)]
            for kc in range(KT):
                stg = wvs_pool.tile([P, DIM], F32, tag="wvs", name=f"wvs{rep}_{kc}")
                nc.sync.dma_start(stg[:], wqkv_d[kc * P : (kc + 1) * P, 2 * DIM : 3 * DIM])
                nc.vector.tensor_copy(w_v[kc][:], stg[:])

            # ------- attention with qk projection drip-fed through jc loop ---
            qk_ps = {}

            def qk_units(ft):
                """Yield thunks: 12 matmuls + 1 eviction for feature tile ft."""

                def alloc():
                    qk_ps[ft] = psB.tile([P, N], F32, tag="psB", name=f"qkp{rep}_{ft}")

                for ih in range(2):
                    for kc in range(KT):

                        def mm(ih=ih, kc=kc, ft=ft):
                            if kc == 0 and ih == 0:
                                alloc()
                            nc.tensor.matmul(
                                qk_ps[ft][:, ih * 512 : (ih + 1) * 512],
                                w_qk[ft][kc][:],
                                x_t[kc][:, ih * 512 : (ih + 1) * 512],
                                start=(kc == 0),
                                stop=(kc == KT - 1),
                            )

                        yield mm

                def evict(ft=ft):
                    nc.vector.tensor_scalar_add(qk_t[ft][:], qk_ps[ft][:], bqk[:, ft : ft + 1])

                yield evict

            # order feature tiles so head h's q (h//2) and k (6+h//2) finish early
            ft_order = []
            for i in range(NH // 2):
                ft_order += [i, NH // 2 + i]
            pending = []

            def drip(n):
                for _ in range(n):
                    if pending:
                        pending.pop(0)()

            # prime: first two feature tiles fully, prefetch DMA for next two
            for u in qk_units(ft_order[0]):
                u()
            for u in qk_units(ft_order[1]):
                u()

            def v_units():
                for tt in range(TT):
                    ps_box = {}

                    def start_tt(tt=tt, ps_box=ps_box):
                        ps_box["ps"] = psB.tile(
                            [P, DIM], F32, tag="psB", name=f"vps{rep}_{tt}"
                        )

                    for ui, (nh0, nh1) in enumerate(((0, 512), (512, 768))):
                        for kc in range(KT):

                            def mm(tt=tt, nh0=nh0, nh1=nh1, kc=kc, ui=ui, ps_box=ps_box, start_tt=start_tt):
                                if ui == 0 and kc == 0:
                                    start_tt()
                                nc.tensor.matmul(
                                    ps_box["ps"][:, nh0:nh1],
                                    x_t[kc][:, tt * P : (tt + 1) * P],
                                    w_v[kc][:, nh0:nh1],
                                    start=(kc == 0),
                                    stop=(kc == KT - 1),
                                )

                            yield mm

                    def evict(tt=tt, ps_box=ps_box):
                        ps = ps_box["ps"]
                        vdst = v_sb[tt][:].rearrange("p (h c) -> p h c", c=HD1)
                        nc.vector.tensor_add(
                            vdst[:, :, 0:HD],
                            ps[:].rearrange("p (h c) -> p h c", c=HD),
                            bv_bc[:].rearrange("p (h c) -> p h c", c=HD),
                        )
                        nc.vector.tensor_scalar(
                            vdst[:, :, HD:HD1],
                            bv_bc[:, 0:NH].rearrange("p (h o) -> p h o", o=1),
                            0.0,
                            1.0,
                            op0=mybir.AluOpType.mult,
                            op1=mybir.AluOpType.add,
                        )

                    yield evict

            pending += list(v_units())
            emit_wdma_pair(1)
            pending += list(qk_units(ft_order[2])) + list(qk_units(ft_order[3]))
            next_pair = 2

            for h in range(NH):
                p0 = HD * (h % 2)
                qt = qk_t[h // 2]
                kt = qk_t[NH // 2 + h // 2]
                po = psB.tile([HD1, N], F32, tag="psB", name=f"po{rep}_{h}")
                pes = [None] * TT

                def av(jc):
                    for ih in range(2):
                        nc.tensor.matmul(
                            po[:, ih * 512 : (ih + 1) * 512],
                            v_sb[jc][:, h * HD1 : (h + 1) * HD1],
                            pes[jc][:, ih * 512 : (ih + 1) * 512],
                            start=(jc == 0),
                            stop=(jc == TT - 1),
                        )

                for jc in range(TT):
                    ps = psA.tile([P, N], F32, tag="psA", name=f"sc{rep}_{h}_{jc}")
                    for ih in range(2):
                        nc.tensor.matmul(
                            ps[:, ih * 512 : (ih + 1) * 512],
                            kt[p0 : p0 + HD, jc * P : (jc + 1) * P],
                            qt[p0 : p0 + HD, ih * 512 : (ih + 1) * 512],
                            start=True,
                            stop=True,
                            tile_position=(p0, 0),
                        )
                    pe = p_pool.tile([P, N], BF16, tag="psb", name=f"pe{rep}_{h}_{jc}")
                    nc.scalar.activation(pe[:], ps[:], EXP, scale=SCALE)
                    pes[jc] = pe
                    # hide qk/v work behind each attention step (v first two
                    # heads need higher drain rate; av trails by one jc)
                    drip(16 if h == 0 else 2)
                    if jc > 0:
                        av(jc - 1)
                av(TT - 1)
                # stage AV psum out to SBUF immediately to release the slot;
                # normalization happens off the critical path
                stg = stg_pool.tile([HD1, N], F32, tag="stg", name=f"stg{rep}_{h}")
                nc.vector.tensor_copy(stg[:], po[:])
                rden = nrm_pool.tile([1, N], F32, tag="rden", name=f"rd{rep}_{h}")
                nc.vector.reciprocal(rden[:], stg[HD:HD1, :])
                rbc = nrm_pool.tile([HD, N], F32, tag="rbc", name=f"rb{rep}_{h}")
                nc.gpsimd.partition_broadcast(rbc[:], rden[:])
                nc.vector.tensor_mul(attn_t[h // 2][p0 : p0 + HD, :], stg[0:HD, :], rbc[:])
                # queue the dma + matmul units for upcoming feature tiles
                if h % 2 == 0 and next_pair < NH // 2:
                    emit_wdma_pair(next_pair)
                    pending += list(qk_units(next_pair)) + list(
                        qk_units(NH // 2 + next_pair)
                    )
                    next_pair += 1
            while pending:
                pending.pop(0)()

        # ---------------- proj ----------------
        with (
            tc.tile_pool(name=f"wp{rep}", bufs=KT) as wp_pool,
            tc.tile_pool(name=f"ysb{rep}", bufs=3) as y_pool,
        ):
            w_p = [wp_pool.tile([P, DIM], BF16, tag="wp", name=f"wp{rep}_{i}") for i in range(KT)]
            for kc in range(KT):
                stg = y_pool.tile([P, DIM], F32, tag="ysb", name=f"wps{rep}_{kc}")
                nc.sync.dma_start(stg[:], wproj_d[kc * P : (kc + 1) * P, :])
                nc.vector.tensor_copy(w_p[kc][:], stg[:])
            for it in range(TT):
                ps = psB.tile([P, DIM], F32, tag="psB", name=f"yps{rep}_{it}")
                for nh0, nh1 in ((0, 512), (512, 768)):
                    for kc in range(KT):
                        nc.tensor.matmul(
                            ps[:, nh0:nh1],
                            attn_t[kc][:, it * P : (it + 1) * P],
                            w_p[kc][:, nh0:nh1],
                            start=(kc == 0),
                            stop=(kc == KT - 1),
                        )
                ysb = y_pool.tile([P, DIM], F32, tag="ysb", name=f"y{rep}_{it}")
                nc.vector.tensor_add(ysb[:], ps[:], bp_bc[:])
                nc.sync.dma_start(out_d[it * P : (it + 1) * P, :], ysb[:])



def build_attention_nc(repeat=1, transpose_mode="pe", p_bufs=4, xq="sp"):
    nc = bacc.Bacc(None, target_bir_lowering=False)
    x_d = nc.declare_dram_parameter("x", [N, DIM], F32, isOutput=False)
    wqkv_d = nc.declare_dram_parameter("w_qkv", [DIM, 3 * DIM], F32, isOutput=False)
    bqkv_d = nc.declare_dram_parameter("b_qkv", [3 * DIM], F32, isOutput=False)
    wproj_d = nc.declare_dram_parameter("w_proj", [DIM, DIM], F32, isOutput=False)
    bproj_d = nc.declare_dram_parameter("b_proj", [DIM], F32, isOutput=False)
    out_d = nc.declare_dram_parameter("out", [N, DIM], F32, isOutput=True)
    dram = (x_d, wqkv_d, bqkv_d, wproj_d, bproj_d, out_d)

    with tile.TileContext(nc) as tc:
        for rep in range(repeat):
            _emit_body(nc, tc, rep, dram, transpose_mode=transpose_mode, p_bufs=int(p_bufs), xq=xq)

    nc.compile()
    return nc


_NC_CACHE = None


def _get_nc():
    global _NC_CACHE
    if _NC_CACHE is None:
        _NC_CACHE = build_attention_nc()
    return _NC_CACHE


def make_in_maps(inputs):
    x = np.ascontiguousarray(np.asarray(inputs["x"], dtype=np.float32))
    w_qkv = np.ascontiguousarray(np.asarray(inputs["w_qkv"], dtype=np.float32))
    b_qkv = np.ascontiguousarray(np.asarray(inputs["b_qkv"], dtype=np.float32))
    w_proj = np.ascontiguousarray(np.asarray(inputs["w_proj"], dtype=np.float32))
    b_proj = np.ascontiguousarray(np.asarray(inputs["b_proj"], dtype=np.float32))
    return [
        {"x": x[c], "w_qkv": w_qkv, "b_qkv": b_qkv, "w_proj": w_proj, "b_proj": b_proj}
        for c in range(N_CORES)
    ]


def kernel(**inputs) -> np.ndarray:
    nc = _get_nc()
    in_maps = make_in_maps(inputs)
    res = run_bass_kernel_spmd(nc, in_maps, core_ids=list(range(N_CORES)))
    return np.stack([res.results[c]["out"] for c in range(N_CORES)], axis=0)


# revision 30
# speedup vs baseline: 50.3493x; 1.3499x over previous
"""Multi-head attention forward (B=8, N=1024, DIM=768, H=12) on 8 TRN2 cores.

Data parallel: core c computes batch element c entirely locally (no
collectives). Layouts chosen so no probability-matrix transpose is needed:
  - q_t, k_t feature-major [dim, tok]  (2 heads per 128-partition tile)
  - v token-major [tok, head*(64+1)] with a ones column per head, so the
    AV matmul's psum row 64 accumulates the softmax denominator
  - scores computed transposed s_T[j, i]; softmax sum over j comes from the
    ones column; exp has no max-subtraction (scores ~ N(0,1), fp32-safe)
  - AV output is feature-major -> feeds proj lhsT directly
All matmul operands are bf16 (4x faster than fp32 on the PE, fast weight
loads); PSUM accumulation stays fp32. The x-transpose runs on the PE in
float32r. The qk-projection and v matmuls are drip-fed through the attention
jc loop so the ScalarE exp stream paces the kernel while the PE hides the
projection work behind it.
"""

from contextlib import ExitStack

import numpy as np

import concourse.bacc as bacc
import concourse.mybir as mybir
import concourse.tile as tile
from concourse.bass_utils import run_bass_kernel_spmd
from concourse.masks import make_identity

N = 1024
DIM = 768
NH = 12
HD = 64
SCALE = HD ** -0.5
P = 128
KT = DIM // P          # 6 contraction tiles over dim
TT = N // P            # 8 token tiles
QKFT = 2 * DIM // P    # 12 feature tiles covering q then k
HD1 = HD + 1           # 65: v head slice + ones column
F32 = mybir.dt.float32
F32R = mybir.dt.float32r
BF16 = mybir.dt.bfloat16
EXP = mybir.ActivationFunctionType.Exp
N_CORES = 8


def _r(ap):
    return ap.bitcast(F32R)


def _emit_body(nc, tc, rep, dram, transpose_mode="pe", p_bufs=4, xq="sp", drip0=26):
    x_d, wqkv_d, bqkv_d, wproj_d, bproj_d, out_d = dram

    with ExitStack() as body:
        const = body.enter_context(tc.tile_pool(name=f"const{rep}", bufs=1))
        # shared PSUM pools (8 banks):
        #  A: 2 x [128,1024] (4KB) slots - transposes, scores psum
        #  B: 2 x 4KB slots - v psum, qk psum, AV psum, proj psum
        psA = body.enter_context(tc.tile_pool(name=f"psA{rep}", bufs=2, space="PSUM"))
        psB = body.enter_context(tc.tile_pool(name=f"psB{rep}", bufs=2, space="PSUM"))

        # resident SBUF intermediates
        sb = body.enter_context(tc.tile_pool(name=f"sb{rep}", bufs=1))
        x_t = [sb.tile([P, N], BF16, name=f"xt{rep}_{i}") for i in range(KT)]
        qk_t = [sb.tile([P, N], BF16, name=f"qkt{rep}_{i}") for i in range(QKFT)]
        v_sb = [sb.tile([P, NH * HD1], BF16, name=f"vsb{rep}_{i}") for i in range(TT)]
        attn_t = [sb.tile([P, N], BF16, name=f"attnt{rep}_{i}") for i in range(KT)]

        if transpose_mode == "dma":
            # ------ x^T off the PE: cast to bf16 in DRAM, DMA-transpose in ---
            with tc.tile_pool(name=f"xbf{rep}", bufs=1, space="DRAM") as xbf_pool:
                xbf = xbf_pool.tile([N, DIM], BF16, name=f"xbf{rep}")
                nc.gpsimd.dma_start(xbf[:], x_d[:])
                for kc in range(KT):
                    nc.sync.dma_start_transpose(
                        x_t[kc][:], xbf[:, kc * P : (kc + 1) * P]
                    )
        else:
            # ------------ x^T on the PE (f32r transpose, bf16 copyback) ------
            ident_f = const.tile([P, P], F32, name=f"identf{rep}")
            make_identity(nc, ident_f[:])
            ident = const.tile([P, P], F32R, name=f"ident{rep}")
            nc.scalar.copy(ident[:], ident_f[:])
            with tc.tile_pool(name=f"xnat{rep}", bufs=3) as xnat_pool:
                for tt in range(TT):
                    xn = xnat_pool.tile([P, DIM], F32R, tag="xn", name=f"xn{rep}_{tt}")
                    xeng = nc.scalar if xq == "act" else nc.sync
                    xeng.dma_start(xn[:], _r(x_d[tt * P : (tt + 1) * P, :]))
                    for kc in range(KT):
                        pt = psA.tile([P, P], F32R, tag="psA", name=f"trp{rep}_{tt}_{kc}")
                        nc.tensor.transpose(pt[:], xn[:, kc * P : (kc + 1) * P], ident[:])
                        nc.vector.tensor_copy(x_t[kc][:, tt * P : (tt + 1) * P], pt[:])

        # biases: q/k per-partition [128, ft]; v and proj broadcast to [128, 768]
        bqk = const.tile([P, QKFT], F32, name=f"bqk{rep}")
        nc.sync.dma_start(bqk[:], bqkv_d[0 : 2 * DIM].rearrange("(f p) -> p f", p=P))
        bv_row = const.tile([1, DIM], F32, name=f"bvr{rep}")
        nc.sync.dma_start(bv_row[:], bqkv_d[2 * DIM : 3 * DIM].rearrange("(a d) -> a d", a=1))
        bv_bc = const.tile([P, DIM], F32, name=f"bvb{rep}")
        nc.gpsimd.partition_broadcast(bv_bc[:], bv_row[:])
        bp_row = const.tile([1, DIM], F32, name=f"bpr{rep}")
        nc.sync.dma_start(bp_row[:], bproj_d[:].rearrange("(a d) -> a d", a=1))
        bp_bc = const.tile([P, DIM], F32, name=f"bpb{rep}")
        nc.gpsimd.partition_broadcast(bp_bc[:], bp_row[:])

        # ------- attention pools (opened early so qk weights prime first) ----
        with (
            tc.tile_pool(name=f"wv{rep}", bufs=KT) as wv_pool,
            tc.tile_pool(name=f"wvs{rep}", bufs=3) as wvs_pool,
            tc.tile_pool(name=f"wqk{rep}", bufs=24) as wqk_pool,
            tc.tile_pool(name=f"wqs{rep}", bufs=12) as wqs_pool,
            tc.tile_pool(name=f"psb{rep}", bufs=p_bufs) as p_pool,
            tc.tile_pool(name=f"nrm{rep}", bufs=2) as nrm_pool,
            tc.tile_pool(name=f"stg{rep}", bufs=2) as stg_pool,
        ):
            w_qk = {}
            for ft in range(QKFT):
                w_qk[ft] = []
                for kc in range(KT):
                    wt = wqk_pool.tile([P, P], BF16, tag="wqk", name=f"wqk{rep}_{ft}_{kc}")
                    w_qk[ft].append(wt)

            def emit_wdma_pair(ftq):
                ftk = NH // 2 + ftq
                for kc in range(KT):
                    stg = wqs_pool.tile(
                        [P, 2, P], F32, tag="wqs", name=f"wqs{rep}_{ftq}_{kc}"
                    )
                    src_ap = wqkv_d[kc * P : (kc + 1) * P, :].rearrange(
                        "p (f c) -> p f c", c=P
                    )[:, ftq :: NH // 2, :][:, 0:2, :]
                    nc.sync.dma_start(stg[:], src_ap)
                    nc.vector.tensor_copy(w_qk[ftq][kc][:], stg[:, 0, :])
                    nc.vector.tensor_copy(w_qk[ftk][kc][:], stg[:, 1, :])

            # qk weights for the first feature-tile pair before w_v traffic
            emit_wdma_pair(0)

            # ---------------- v = x @ w_v (token-major + ones col) ------------
            w_v = [wv_pool.tile([P, DIM], BF16, tag="wv", name=f"wv{rep}_{i}") for i in range(KT)]
            for kc in range(KT):
                stg = wvs_pool.tile([P, DIM], F32, tag="wvs", name=f"wvs{rep}_{kc}")
                nc.sync.dma_start(stg[:], wqkv_d[kc * P : (kc + 1) * P, 2 * DIM : 3 * DIM])
                nc.vector.tensor_copy(w_v[kc][:], stg[:])

            # ------- attention with qk projection drip-fed through jc loop ---
            qk_ps = {}

            def qk_units(ft):
                """Yield thunks: 12 matmuls + 1 eviction for feature tile ft."""

                def alloc():
                    qk_ps[ft] = psB.tile([P, N], F32, tag="psB", name=f"qkp{rep}_{ft}")

                for ih in range(2):
                    for kc in range(KT):

                        def mm(ih=ih, kc=kc, ft=ft):
                            if kc == 0 and ih == 0:
                                alloc()
                            nc.tensor.matmul(
                                qk_ps[ft][:, ih * 512 : (ih + 1) * 512],
                                w_qk[ft][kc][:],
                                x_t[kc][:, ih * 512 : (ih + 1) * 512],
                                start=(kc == 0),
                                stop=(kc == KT - 1),
                            )

                        yield mm

                def evict(ft=ft):
                    nc.vector.tensor_scalar_add(qk_t[ft][:], qk_ps[ft][:], bqk[:, ft : ft + 1])

                yield evict

            # order feature tiles so head h's q (h//2) and k (6+h//2) finish early
            ft_order = []
            for i in range(NH // 2):
                ft_order += [i, NH // 2 + i]
            pending = []

            def drip(n):
                for _ in range(n):
                    if pending:
                        pending.pop(0)()

            # prime: first two feature tiles fully, prefetch DMA for next two
            for u in qk_units(ft_order[0]):
                u()
            for u in qk_units(ft_order[1]):
                u()

            def v_units():
                for tt in range(TT):
                    ps_box = {}

                    def start_tt(tt=tt, ps_box=ps_box):
                        ps_box["ps"] = psB.tile(
                            [P, DIM], F32, tag="psB", name=f"vps{rep}_{tt}"
                        )

                    for ui, (nh0, nh1) in enumerate(((0, 512), (512, 768))):
                        for kc in range(KT):

                            def mm(tt=tt, nh0=nh0, nh1=nh1, kc=kc, ui=ui, ps_box=ps_box, start_tt=start_tt):
                                if ui == 0 and kc == 0:
                                    start_tt()
                                nc.tensor.matmul(
                                    ps_box["ps"][:, nh0:nh1],
                                    x_t[kc][:, tt * P : (tt + 1) * P],
                                    w_v[kc][:, nh0:nh1],
                                    start=(kc == 0),
                                    stop=(kc == KT - 1),
                                )

                            yield mm

                    def evict(tt=tt, ps_box=ps_box):
                        ps = ps_box["ps"]
                        vdst = v_sb[tt][:].rearrange("p (h c) -> p h c", c=HD1)
                        nc.vector.tensor_add(
                            vdst[:, :, 0:HD],
                            ps[:].rearrange("p (h c) -> p h c", c=HD),
                            bv_bc[:].rearrange("p (h c) -> p h c", c=HD),
                        )
                        nc.vector.tensor_scalar(
                            vdst[:, :, HD:HD1],
                            bv_bc[:, 0:NH].rearrange("p (h o) -> p h o", o=1),
                            0.0,
                            1.0,
                            op0=mybir.AluOpType.mult,
                            op1=mybir.AluOpType.add,
                        )

                    yield evict

            pending += list(v_units())
            emit_wdma_pair(1)
            pending += list(qk_units(ft_order[2])) + list(qk_units(ft_order[3]))
            next_pair = 2

            for h in range(NH):
                p0 = HD * (h % 2)
                qt = qk_t[h // 2]
                kt = qk_t[NH // 2 + h // 2]
                po = psB.tile([HD1, N], F32, tag="psB", name=f"po{rep}_{h}")
                pes = [None] * TT

                def av(jc):
                    for ih in range(2):
                        nc.tensor.matmul(
                            po[:, ih * 512 : (ih + 1) * 512],
                            v_sb[jc][:, h * HD1 : (h + 1) * HD1],
                            pes[jc][:, ih * 512 : (ih + 1) * 512],
                            start=(jc == 0),
                            stop=(jc == TT - 1),
                        )

                for jc in range(TT):
                    ps = psA.tile([P, N], F32, tag="psA", name=f"sc{rep}_{h}_{jc}")
                    for ih in range(2):
                        nc.tensor.matmul(
                            ps[:, ih * 512 : (ih + 1) * 512],
                            kt[p0 : p0 + HD, jc * P : (jc + 1) * P],
                            qt[p0 : p0 + HD, ih * 512 : (ih + 1) * 512],
                            start=True,
                            stop=True,
                            tile_position=(p0, 0),
                        )
                    pe = p_pool.tile([P, N], BF16, tag="psb", name=f"pe{rep}_{h}_{jc}")
                    nc.scalar.activation(pe[:], ps[:], EXP, scale=SCALE)
                    pes[jc] = pe
                    # hide qk/v work behind each attention step (v first two
                    # heads need higher drain rate; av trails by one jc)
                    drip(drip0 if h == 0 else 2)
                    if jc > 0:
                        av(jc - 1)
                av(TT - 1)
                # stage AV psum out to SBUF immediately to release the slot;
                # normalization happens off the critical path
                stg = stg_pool.tile([HD1, N], F32, tag="stg", name=f"stg{rep}_{h}")
                nc.vector.tensor_copy(stg[:], po[:])
                rden = nrm_pool.tile([1, N], F32, tag="rden", name=f"rd{rep}_{h}")
                nc.vector.reciprocal(rden[:], stg[HD:HD1, :])
                rbc = nrm_pool.tile([HD, N], F32, tag="rbc", name=f"rb{rep}_{h}")
                nc.gpsimd.partition_broadcast(rbc[:], rden[:])
                nc.vector.tensor_mul(attn_t[h // 2][p0 : p0 + HD, :], stg[0:HD, :], rbc[:])
                # queue the dma + matmul units for upcoming feature tiles
                if h % 2 == 0 and next_pair < NH // 2:
                    emit_wdma_pair(next_pair)
                    pending += list(qk_units(next_pair)) + list(
                        qk_units(NH // 2 + next_pair)
                    )
                    next_pair += 1
            while pending:
                pending.pop(0)()

        # ---------------- proj ----------------
        with (
            tc.tile_pool(name=f"wp{rep}", bufs=KT) as wp_pool,
            tc.tile_pool(name=f"ysb{rep}", bufs=3) as y_pool,
        ):
            w_p = [wp_pool.tile([P, DIM], BF16, tag="wp", name=f"wp{rep}_{i}") for i in range(KT)]
            for kc in range(KT):
                stg = y_pool.tile([P, DIM], F32, tag="ysb", name=f"wps{rep}_{kc}")
                nc.sync.dma_start(stg[:], wproj_d[kc * P : (kc + 1) * P, :])
                nc.vector.tensor_copy(w_p[kc][:], stg[:])
            for it in range(TT):
                ps = psB.tile([P, DIM], F32, tag="psB", name=f"yps{rep}_{it}")
                for nh0, nh1 in ((0, 512), (512, 768)):
                    for kc in range(KT):
                        nc.tensor.matmul(
                            ps[:, nh0:nh1],
                            attn_t[kc][:, it * P : (it + 1) * P],
                            w_p[kc][:, nh0:nh1],
                            start=(kc == 0),
                            stop=(kc == KT - 1),
                        )
                ysb = y_pool.tile([P, DIM], F32, tag="ysb", name=f"y{rep}_{it}")
                nc.vector.tensor_add(ysb[:], ps[:], bp_bc[:])
                nc.sync.dma_start(out_d[it * P : (it + 1) * P, :], ysb[:])



def build_attention_nc(repeat=1, transpose_mode="pe", p_bufs=4, xq="sp", drip0=26):
    nc = bacc.Bacc(None, target_bir_lowering=False)
    x_d = nc.declare_dram_parameter("x", [N, DIM], F32, isOutput=False)
    wqkv_d = nc.declare_dram_parameter("w_qkv", [DIM, 3 * DIM], F32, isOutput=False)
    bqkv_d = nc.declare_dram_parameter("b_qkv", [3 * DIM], F32, isOutput=False)
    wproj_d = nc.declare_dram_parameter("w_proj", [DIM, DIM], F32, isOutput=False)
    bproj_d = nc.declare_dram_parameter("b_proj", [DIM], F32, isOutput=False)
    out_d = nc.declare_dram_parameter("out", [N, DIM], F32, isOutput=True)
    dram = (x_d, wqkv_d, bqkv_d, wproj_d, bproj_d, out_d)

    with tile.TileContext(nc) as tc:
        for rep in range(repeat):
            _emit_body(nc, tc, rep, dram, transpose_mode=transpose_mode, p_bufs=int(p_bufs), xq=xq, drip0=int(drip0))

    nc.compile()
    return nc


_NC_CACHE = None


def _get_nc():
    global _NC_CACHE
    if _NC_CACHE is None:
        _NC_CACHE = build_attention_nc()
    return _NC_CACHE


def make_in_maps(inputs):
    x = np.ascontiguousarray(np.asarray(inputs["x"], dtype=np.float32))
    w_qkv = np.ascontiguousarray(np.asarray(inputs["w_qkv"], dtype=np.float32))
    b_qkv = np.ascontiguousarray(np.asarray(inputs["b_qkv"], dtype=np.float32))
    w_proj = np.ascontiguousarray(np.asarray(inputs["w_proj"], dtype=np.float32))
    b_proj = np.ascontiguousarray(np.asarray(inputs["b_proj"], dtype=np.float32))
    return [
        {"x": x[c], "w_qkv": w_qkv, "b_qkv": b_qkv, "w_proj": w_proj, "b_proj": b_proj}
        for c in range(N_CORES)
    ]


def kernel(**inputs) -> np.ndarray:
    nc = _get_nc()
    in_maps = make_in_maps(inputs)
    res = run_bass_kernel_spmd(nc, in_maps, core_ids=list(range(N_CORES)))
    return np.stack([res.results[c]["out"] for c in range(N_CORES)], axis=0)


# revision 34
# speedup vs baseline: 114.7163x; 2.2784x over previous
"""Multi-head attention forward (B=8, N=1024, DIM=768, H=12) on 8 TRN2 cores.

Data parallel: core c computes batch element c entirely locally (no
collectives). Layouts chosen so no probability-matrix transpose is needed:
  - q_t, k_t feature-major [dim, tok]  (2 heads per 128-partition tile)
  - v token-major [tok, head*(64+1)] with a ones column per head, so the
    AV matmul's psum row 64 accumulates the softmax denominator
  - scores computed transposed s_T[j, i]; softmax sum over j comes from the
    ones column; exp has no max-subtraction (scores ~ N(0,1), fp32-safe)
  - AV output is feature-major -> feeds proj lhsT directly
All matmul operands are bf16 (4x faster than fp32 on the PE, fast weight
loads); PSUM accumulation stays fp32. The x-transpose runs on the PE in
float32r. The qk-projection and v matmuls are drip-fed through the attention
jc loop so the ScalarE exp stream paces the kernel while the PE hides the
projection work behind it.
"""

from contextlib import ExitStack

import numpy as np

import concourse.bacc as bacc
import concourse.mybir as mybir
import concourse.tile as tile
from concourse.bass_utils import run_bass_kernel_spmd
from concourse.masks import make_identity

N = 1024
DIM = 768
NH = 12
HD = 64
SCALE = HD ** -0.5
P = 128
KT = DIM // P          # 6 contraction tiles over dim
TT = N // P            # 8 token tiles
QKFT = 2 * DIM // P    # 12 feature tiles covering q then k
HD1 = HD + 1           # 65: v head slice + ones column
F32 = mybir.dt.float32
F32R = mybir.dt.float32r
BF16 = mybir.dt.bfloat16
EXP = mybir.ActivationFunctionType.Exp
N_CORES = 8


def _r(ap):
    return ap.bitcast(F32R)


def _emit_body(nc, tc, rep, dram, transpose_mode="pe", p_bufs=4, xq="sp", drip0=26):
    x_d, wqkv_d, bqkv_d, wproj_d, bproj_d, out_d = dram

    with ExitStack() as body:
        const = body.enter_context(tc.tile_pool(name=f"const{rep}", bufs=1))
        # shared PSUM pools (8 banks):
        #  A: 2 x [128,1024] (4KB) slots - transposes, scores psum
        #  B: 2 x 4KB slots - v psum, qk psum, AV psum, proj psum
        psA = body.enter_context(tc.tile_pool(name=f"psA{rep}", bufs=2, space="PSUM"))
        psB = body.enter_context(tc.tile_pool(name=f"psB{rep}", bufs=2, space="PSUM"))

        # resident SBUF intermediates
        sb = body.enter_context(tc.tile_pool(name=f"sb{rep}", bufs=1))
        x_t_big = sb.tile([P, KT * N], BF16, name=f"xtb{rep}")
        x_t = [x_t_big[:, kc * N : (kc + 1) * N] for kc in range(KT)]
        qk_t = [sb.tile([P, N], BF16, name=f"qkt{rep}_{i}") for i in range(QKFT)]
        v_sb = [sb.tile([P, NH * HD1], BF16, name=f"vsb{rep}_{i}") for i in range(TT)]
        attn_t = [sb.tile([P, N], BF16, name=f"attnt{rep}_{i}") for i in range(KT)]

        def do_transpose():
            if transpose_mode == "dma":
                with tc.tile_pool(name=f"xbf{rep}", bufs=1, space="DRAM") as xbf_pool:
                    xbf = xbf_pool.tile([N, DIM], BF16, name=f"xbf{rep}")
                    nc.gpsimd.dma_start(xbf[:], x_d[:])
                    for kc in range(KT):
                        nc.sync.dma_start_transpose(
                            x_t[kc][:], xbf[:, kc * P : (kc + 1) * P]
                        )
                return
            # x^T on the PE (f32r transpose, bf16 copyback)
            ident_f = const.tile([P, P], F32, name=f"identf{rep}")
            make_identity(nc, ident_f[:])
            ident = const.tile([P, P], F32R, name=f"ident{rep}")
            nc.scalar.copy(ident[:], ident_f[:])
            with tc.tile_pool(name=f"xnat{rep}", bufs=3) as xnat_pool:
                for tt in range(TT):
                    xn = xnat_pool.tile([P, DIM], F32R, tag="xn", name=f"xn{rep}_{tt}")
                    xeng = nc.scalar if xq == "act" else nc.sync
                    xeng.dma_start(xn[:], _r(x_d[tt * P : (tt + 1) * P, :]))
                    # all 6 transposes of this token tile share one psum tile so
                    # the PE never stalls on per-op slot roundtrips
                    pt = psA.tile([P, DIM], F32R, tag="psA", name=f"trp{rep}_{tt}")
                    for kc in range(KT):
                        nc.tensor.transpose(
                            pt[:, kc * P : (kc + 1) * P], xn[:, kc * P : (kc + 1) * P], ident[:]
                        )
                    nc.vector.tensor_copy(
                        x_t_big[:].rearrange("p (k n) -> p k n", k=KT)[
                            :, :, tt * P : (tt + 1) * P
                        ],
                        pt[:].rearrange("p (k c) -> p k c", k=KT),
                    )

        # ------- attention pools (opened early so qk weights prime first) ----
        with (
            tc.tile_pool(name=f"wv{rep}", bufs=KT) as wv_pool,
            tc.tile_pool(name=f"wvs{rep}", bufs=3) as wvs_pool,
            tc.tile_pool(name=f"wqk{rep}", bufs=24) as wqk_pool,
            tc.tile_pool(name=f"wqs{rep}", bufs=12) as wqs_pool,
            tc.tile_pool(name=f"psb{rep}", bufs=p_bufs) as p_pool,
            tc.tile_pool(name=f"nrm{rep}", bufs=2) as nrm_pool,
            tc.tile_pool(name=f"stg{rep}", bufs=2) as stg_pool,
        ):
            w_qk = {}
            for ft in range(QKFT):
                w_qk[ft] = []
                for kc in range(KT):
                    wt = wqk_pool.tile([P, P], BF16, tag="wqk", name=f"wqk{rep}_{ft}_{kc}")
                    w_qk[ft].append(wt)

            def emit_wdma_pair(ftq):
                ftk = NH // 2 + ftq
                for kc in range(KT):
                    stg = wqs_pool.tile(
                        [P, 2, P], F32, tag="wqs", name=f"wqs{rep}_{ftq}_{kc}"
                    )
                    src_ap = wqkv_d[kc * P : (kc + 1) * P, :].rearrange(
                        "p (f c) -> p f c", c=P
                    )[:, ftq :: NH // 2, :][:, 0:2, :]
                    nc.sync.dma_start(stg[:], src_ap)
                    nc.vector.tensor_copy(w_qk[ftq][kc][:], stg[:, 0, :])
                    nc.vector.tensor_copy(w_qk[ftk][kc][:], stg[:, 1, :])

            # x transposes first (they gate everything), then biases + the
            # first qk weight pair
            do_transpose()
            bqk = const.tile([P, QKFT], F32, name=f"bqk{rep}")
            nc.sync.dma_start(bqk[:], bqkv_d[0 : 2 * DIM].rearrange("(f p) -> p f", p=P))
            bv_row = const.tile([1, DIM], F32, name=f"bvr{rep}")
            nc.sync.dma_start(bv_row[:], bqkv_d[2 * DIM : 3 * DIM].rearrange("(a d) -> a d", a=1))
            bv_bc = const.tile([P, DIM], F32, name=f"bvb{rep}")
            nc.gpsimd.partition_broadcast(bv_bc[:], bv_row[:])
            bp_row = const.tile([1, DIM], F32, name=f"bpr{rep}")
            nc.sync.dma_start(bp_row[:], bproj_d[:].rearrange("(a d) -> a d", a=1))
            bp_bc = const.tile([P, DIM], F32, name=f"bpb{rep}")
            nc.gpsimd.partition_broadcast(bp_bc[:], bp_row[:])
            emit_wdma_pair(0)

            # ---------------- v = x @ w_v (token-major + ones col) ------------
            w_v = [wv_pool.tile([P, DIM], BF16, tag="wv", name=f"wv{rep}_{i}") for i in range(KT)]
            for kc in range(KT):
                stg = wvs_pool.tile([P, DIM], F32, tag="wvs", name=f"wvs{rep}_{kc}")
                nc.sync.dma_start(stg[:], wqkv_d[kc * P : (kc + 1) * P, 2 * DIM : 3 * DIM])
                nc.vector.tensor_copy(w_v[kc][:], stg[:])

            # ------- attention with qk projection drip-fed through jc loop ---
            qk_ps = {}

            def qk_units(ft):
                """Yield thunks: 12 matmuls + 1 eviction for feature tile ft."""

                def alloc():
                    qk_ps[ft] = psB.tile([P, N], F32, tag="psB", name=f"qkp{rep}_{ft}")

                for ih in range(2):
                    for kc in range(KT):

                        def mm(ih=ih, kc=kc, ft=ft):
                            if kc == 0 and ih == 0:
                                alloc()
                            nc.tensor.matmul(
                                qk_ps[ft][:, ih * 512 : (ih + 1) * 512],
                                w_qk[ft][kc][:],
                                x_t[kc][:, ih * 512 : (ih + 1) * 512],
                                start=(kc == 0),
                                stop=(kc == KT - 1),
                            )

                        yield mm

                def evict(ft=ft):
                    nc.vector.tensor_scalar_add(qk_t[ft][:], qk_ps[ft][:], bqk[:, ft : ft + 1])

                yield evict

            # order feature tiles so head h's q (h//2) and k (6+h//2) finish early
            ft_order = []
            for i in range(NH // 2):
                ft_order += [i, NH // 2 + i]
            pending = []

            def drip(n):
                for _ in range(n):
                    if pending:
                        pending.pop(0)()

            # prime: first two feature tiles fully, prefetch DMA for next two
            for u in qk_units(ft_order[0]):
                u()
            for u in qk_units(ft_order[1]):
                u()

            def v_units():
                for tt in range(TT):
                    ps_box = {}

                    def start_tt(tt=tt, ps_box=ps_box):
                        ps_box["ps"] = psB.tile(
                            [P, DIM], F32, tag="psB", name=f"vps{rep}_{tt}"
                        )

                    for ui, (nh0, nh1) in enumerate(((0, 512), (512, 768))):
                        for kc in range(KT):

                            def mm(tt=tt, nh0=nh0, nh1=nh1, kc=kc, ui=ui, ps_box=ps_box, start_tt=start_tt):
                                if ui == 0 and kc == 0:
                                    start_tt()
                                nc.tensor.matmul(
                                    ps_box["ps"][:, nh0:nh1],
                                    x_t[kc][:, tt * P : (tt + 1) * P],
                                    w_v[kc][:, nh0:nh1],
                                    start=(kc == 0),
                                    stop=(kc == KT - 1),
                                )

                            yield mm

                    def evict(tt=tt, ps_box=ps_box):
                        ps = ps_box["ps"]
                        vdst = v_sb[tt][:].rearrange("p (h c) -> p h c", c=HD1)
                        nc.vector.tensor_add(
                            vdst[:, :, 0:HD],
                            ps[:].rearrange("p (h c) -> p h c", c=HD),
                            bv_bc[:].rearrange("p (h c) -> p h c", c=HD),
                        )
                        nc.vector.tensor_scalar(
                            vdst[:, :, HD:HD1],
                            bv_bc[:, 0:NH].rearrange("p (h o) -> p h o", o=1),
                            0.0,
                            1.0,
                            op0=mybir.AluOpType.mult,
                            op1=mybir.AluOpType.add,
                        )

                    yield evict

            pending += list(v_units())
            emit_wdma_pair(1)
            pending += list(qk_units(ft_order[2])) + list(qk_units(ft_order[3]))
            next_pair = 2

            for h in range(NH):
                p0 = HD * (h % 2)
                qt = qk_t[h // 2]
                kt = qk_t[NH // 2 + h // 2]
                po = psB.tile([HD1, N], F32, tag="psB", name=f"po{rep}_{h}")
                pes = [None] * TT

                def av(jc):
                    for ih in range(2):
                        nc.tensor.matmul(
                            po[:, ih * 512 : (ih + 1) * 512],
                            v_sb[jc][:, h * HD1 : (h + 1) * HD1],
                            pes[jc][:, ih * 512 : (ih + 1) * 512],
                            start=(jc == 0),
                            stop=(jc == TT - 1),
                        )

                for jc in range(TT):
                    ps = psA.tile([P, N], F32, tag="psA", name=f"sc{rep}_{h}_{jc}")
                    for ih in range(2):
                        nc.tensor.matmul(
                            ps[:, ih * 512 : (ih + 1) * 512],
                            kt[p0 : p0 + HD, jc * P : (jc + 1) * P],
                            qt[p0 : p0 + HD, ih * 512 : (ih + 1) * 512],
                            start=True,
                            stop=True,
                            tile_position=(p0, 0),
                        )
                    pe = p_pool.tile([P, N], BF16, tag="psb", name=f"pe{rep}_{h}_{jc}")
                    nc.scalar.activation(pe[:], ps[:], EXP, scale=SCALE)
                    pes[jc] = pe
                    # hide qk/v work behind each attention step (v first two
                    # heads need higher drain rate; av trails by one jc)
                    drip(drip0 if h == 0 else 2)
                    if jc > 0:
                        av(jc - 1)
                av(TT - 1)
                # stage AV psum out to SBUF immediately to release the slot;
                # normalization happens off the critical path
                stg = stg_pool.tile([HD1, N], F32, tag="stg", name=f"stg{rep}_{h}")
                nc.vector.tensor_copy(stg[:], po[:])
                rden = nrm_pool.tile([1, N], F32, tag="rden", name=f"rd{rep}_{h}")
                nc.vector.reciprocal(rden[:], stg[HD:HD1, :])
                rbc = nrm_pool.tile([HD, N], F32, tag="rbc", name=f"rb{rep}_{h}")
                nc.gpsimd.partition_broadcast(rbc[:], rden[:])
                nc.vector.tensor_mul(attn_t[h // 2][p0 : p0 + HD, :], stg[0:HD, :], rbc[:])
                # queue the dma + matmul units for upcoming feature tiles
                if h % 2 == 0 and next_pair < NH // 2:
                    emit_wdma_pair(next_pair)
                    pending += list(qk_units(next_pair)) + list(
                        qk_units(NH // 2 + next_pair)
                    )
                    next_pair += 1
            while pending:
                pending.pop(0)()

        # ---------------- proj ----------------
        with (
            tc.tile_pool(name=f"wp{rep}", bufs=KT) as wp_pool,
            tc.tile_pool(name=f"ysb{rep}", bufs=3) as y_pool,
        ):
            w_p = [wp_pool.tile([P, DIM], BF16, tag="wp", name=f"wp{rep}_{i}") for i in range(KT)]
            for kc in range(KT):
                stg = y_pool.tile([P, DIM], F32, tag="ysb", name=f"wps{rep}_{kc}")
                nc.sync.dma_start(stg[:], wproj_d[kc * P : (kc + 1) * P, :])
                nc.vector.tensor_copy(w_p[kc][:], stg[:])
            for it in range(TT):
                ps = psB.tile([P, DIM], F32, tag="psB", name=f"yps{rep}_{it}")
                for nh0, nh1 in ((0, 512), (512, 768)):
                    for kc in range(KT):
                        nc.tensor.matmul(
                            ps[:, nh0:nh1],
                            attn_t[kc][:, it * P : (it + 1) * P],
                            w_p[kc][:, nh0:nh1],
                            start=(kc == 0),
                            stop=(kc == KT - 1),
                        )
                ysb = y_pool.tile([P, DIM], F32, tag="ysb", name=f"y{rep}_{it}")
                nc.vector.tensor_add(ysb[:], ps[:], bp_bc[:])
                nc.sync.dma_start(out_d[it * P : (it + 1) * P, :], ysb[:])



def build_attention_nc(repeat=1, transpose_mode="pe", p_bufs=4, xq="sp", drip0=26):
    nc = bacc.Bacc(None, target_bir_lowering=False)
    x_d = nc.declare_dram_parameter("x", [N, DIM], F32, isOutput=False)
    wqkv_d = nc.declare_dram_parameter("w_qkv", [DIM, 3 * DIM], F32, isOutput=False)
    bqkv_d = nc.declare_dram_parameter("b_qkv", [3 * DIM], F32, isOutput=False)
    wproj_d = nc.declare_dram_parameter("w_proj", [DIM, DIM], F32, isOutput=False)
    bproj_d = nc.declare_dram_parameter("b_proj", [DIM], F32, isOutput=False)
    out_d = nc.declare_dram_parameter("out", [N, DIM], F32, isOutput=True)
    dram = (x_d, wqkv_d, bqkv_d, wproj_d, bproj_d, out_d)

    with tile.TileContext(nc) as tc:
        for rep in range(repeat):
            _emit_body(nc, tc, rep, dram, transpose_mode=transpose_mode, p_bufs=int(p_bufs), xq=xq, drip0=int(drip0))

    nc.compile()
    return nc


_NC_CACHE = None


def _get_nc():
    global _NC_CACHE
    if _NC_CACHE is None:
        _NC_CACHE = build_attention_nc()
    return _NC_CACHE


def make_in_maps(inputs):
    x = np.ascontiguousarray(np.asarray(inputs["x"], dtype=np.float32))
    w_qkv = np.ascontiguousarray(np.asarray(inputs["w_qkv"], dtype=np.float32))
    b_qkv = np.ascontiguousarray(np.asarray(inputs["b_qkv"], dtype=np.float32))
    w_proj = np.ascontiguousarray(np.asarray(inputs["w_proj"], dtype=np.float32))
    b_proj = np.ascontiguousarray(np.asarray(inputs["b_proj"], dtype=np.float32))
    return [
        {"x": x[c], "w_qkv": w_qkv, "b_qkv": b_qkv, "w_proj": w_proj, "b_proj": b_proj}
        for c in range(N_CORES)
    ]


def kernel(**inputs) -> np.ndarray:
    nc = _get_nc()
    in_maps = make_in_maps(inputs)
    res = run_bass_kernel_spmd(nc, in_maps, core_ids=list(range(N_CORES)))
    return np.stack([res.results[c]["out"] for c in range(N_CORES)], axis=0)
